# revision 3
# baseline (speedup 1.0000x reference)
"""Trainium2 Bass kernel for nn_CTRule (temporal KG scoring model).

Computes, for each of B=1024 queries (h, r, t):
  v = f(E0[h], E1[r], time tables, rule tables)   # [B, 128] elementwise algebra
  scores = v @ E0.T                               # [B, 40000]

Sharding: E0's 40000-entity axis is split column-wise across 8 NeuronCores
(5000 entities each, model parallel); the small relation/time tables and the
batch are replicated.  Each core gathers the per-example embedding rows with
indirect DMA, runs the elementwise algebra on VectorE, and matmuls against its
pre-transposed bf16 E0 shard on TensorE, emitting a [1024, 5000] f32 slice.
"""

import numpy as np
import ml_dtypes

P = 128
B = 1024
RANK = 128
NENT = 40000
NREL = 230
NTIME = 365
NBASE = 4
CYCLE = 120
NCORES = 8
NSHARD = NENT // NCORES  # 5000
NT = B // P              # 8 batch tiles
CHUNK = 512              # PSUM bank width in f32

# concatenated-table row widths
RC_W = 2 * RANK + 2      # [E1 | rule_C | rule_S | has_rules] = 258
TC_W = 3 * RANK          # [E4 | E2 | E3] = 384
BC_W = 2 * RANK          # [E5 | E6] = 256

GT = 4                   # batch tiles per elementwise group

TRACE = False            # set by test harness for profiling runs
_CACHE = {}


def _build():
    import concourse.bass as bass
    import concourse.mybir as mybir
    import concourse.tile as tile
    from concourse import bacc
    from concourse.masks import make_identity

    dt = mybir.dt
    mult = mybir.AluOpType.mult
    add = mybir.AluOpType.add
    sub = mybir.AluOpType.subtract

    nc = bacc.Bacc("TRN2", target_bir_lowering=False, debug=False,
                   num_devices=NCORES)

    IDX = nc.dram_tensor("IDX", [B, 4], dt.int32, kind="ExternalInput").ap()
    E0 = nc.dram_tensor("E0", [NENT, RANK], dt.float32, kind="ExternalInput").ap()
    RCAT = nc.dram_tensor("RCAT", [NREL, RC_W], dt.float32, kind="ExternalInput").ap()
    TCAT = nc.dram_tensor("TCAT", [NTIME, TC_W], dt.float32, kind="ExternalInput").ap()
    BCAT = nc.dram_tensor("BCAT", [NBASE, BC_W], dt.float32, kind="ExternalInput").ap()
    E0T = nc.dram_tensor("E0T", [RANK, NSHARD], dt.bfloat16, kind="ExternalInput").ap()
    OUT = nc.dram_tensor("OUT", [B, NSHARD], dt.float32, kind="ExternalOutput").ap()

    with tile.TileContext(nc) as tc:
        with (
            tc.tile_pool(name="const", bufs=1) as constp,
            tc.tile_pool(name="gath", bufs=2) as gp,
            tc.tile_pool(name="ew", bufs=2) as ew,
            tc.tile_pool(name="vt", bufs=4) as vtp,
            tc.tile_pool(name="outp", bufs=2) as outp,
            tc.tile_pool(name="pst", bufs=2, space="PSUM") as pst,
            tc.tile_pool(name="psm", bufs=4, space="PSUM") as psm,
        ):
            ident = constp.tile([P, P], dt.float32)
            make_identity(nc, ident[:])
            e0t = constp.tile([RANK, NSHARD], dt.bfloat16)
            nc.sync.dma_start(e0t[:], E0T[:])

            def TT(out, a, b_, op):
                nc.vector.tensor_tensor(out=out, in0=a, in1=b_, op=op)

            for g in range(NT // GT):
                # ---- gather phase: one IDX DMA + 4 indirect gathers per tile
                lhs8 = gp.tile([P, GT, RANK], dt.float32, tag="lhs8")
                r8 = gp.tile([P, GT, RC_W], dt.float32, tag="r8")
                t8 = gp.tile([P, GT, TC_W], dt.float32, tag="t8")
                b8 = gp.tile([P, GT, BC_W], dt.float32, tag="b8")
                for j in range(GT):
                    bt = g * GT + j
                    idxt = gp.tile([P, 4], dt.int32, tag="idx", bufs=2 * GT)
                    nc.sync.dma_start(idxt[:], IDX[bt * P:(bt + 1) * P, :])
                    for dst, src, col in (
                        (lhs8, E0, 0), (r8, RCAT, 1), (t8, TCAT, 2), (b8, BCAT, 3),
                    ):
                        nc.gpsimd.indirect_dma_start(
                            out=dst[:, j, :],
                            out_offset=None,
                            in_=src[:],
                            in_offset=bass.IndirectOffsetOnAxis(
                                ap=idxt[:, col:col + 1], axis=0),
                        )

                # ---- named slices of the gathered mega-tiles
                REL = r8[:, :, 0:128]
                RL0, RL1 = r8[:, :, 0:64], r8[:, :, 64:128]
                RC0, RC1 = r8[:, :, 128:192], r8[:, :, 192:256]
                RS = r8[:, :, 256:257].to_broadcast([P, GT, RANK])
                HR = r8[:, :, 257:258].to_broadcast([P, GT, RANK])
                CT = t8[:, :, 0:128]
                CT0, CT1 = t8[:, :, 0:64], t8[:, :, 64:128]
                E2g, B5 = t8[:, :, 128:256], b8[:, :, 0:128]
                E3g, B6 = t8[:, :, 256:384], b8[:, :, 128:256]
                LHS = lhs8[:, :, 0:128]
                L0, L1 = lhs8[:, :, 0:64], lhs8[:, :, 64:128]

                TM = ew.tile([P, GT, RANK], dt.float32, tag="TM")
                TE = ew.tile([P, GT, RANK], dt.float32, tag="TE")
                A = ew.tile([P, GT, RANK], dt.float32, tag="A")
                Bt = ew.tile([P, GT, RANK], dt.float32, tag="Bt")
                Sg = ew.tile([P, GT, RANK], dt.float32, tag="Sg")
                Dg = ew.tile([P, GT, RANK], dt.float32, tag="Dg")
                V = ew.tile([P, GT, RANK], dt.float32, tag="V")
                t0 = ew.tile([P, GT, 64], dt.float32, tag="t0")
                t1 = ew.tile([P, GT, 64], dt.float32, tag="t1")

                def h0(x):
                    return x[:, :, 0:64]

                def h1(x):
                    return x[:, :, 64:128]

                # time = E2[t] + E5[tb];  time_ent = E3[t] + E6[tb]
                TT(TM[:], E2g, B5, add)
                TT(TE[:], E3g, B6, add)
                # A = cmul(comp_time, rule_C[r])
                TT(t0[:], CT0, RC0, mult)
                TT(t1[:], CT1, RC1, mult)
                TT(h0(A), t0[:], t1[:], sub)
                TT(t0[:], CT0, RC1, mult)
                TT(t1[:], CT1, RC0, mult)
                TT(h1(A), t0[:], t1[:], add)
                # A = rule_branch = A - rule_S * rel
                TT(Bt[:], REL, RS, mult)
                TT(A[:], A[:], Bt[:], sub)
                # Bt = cmul(rel, lhs); then Bt = norule_branch = lhs + Bt
                TT(t0[:], RL0, L0, mult)
                TT(t1[:], RL1, L1, mult)
                TT(h0(Bt), t0[:], t1[:], sub)
                TT(t0[:], RL0, L1, mult)
                TT(t1[:], RL1, L0, mult)
                TT(h1(Bt), t0[:], t1[:], add)
                TT(Bt[:], LHS, Bt[:], add)
                # A = rule_score = Bt + HR*(A - Bt);  then A = q = comp_time + A
                TT(A[:], A[:], Bt[:], sub)
                TT(A[:], A[:], HR, mult)
                TT(A[:], A[:], Bt[:], add)
                TT(A[:], A[:], CT, add)
                # Bt = complex_mul(rel, q) = [R0*q0 + R1*q1, R0*q1 - R1*q0]
                TT(t0[:], RL0, h0(A), mult)
                TT(t1[:], RL1, h1(A), mult)
                TT(h0(Bt), t0[:], t1[:], add)
                TT(t0[:], RL0, h1(A), mult)
                TT(t1[:], RL1, h0(A), mult)
                TT(h1(Bt), t0[:], t1[:], sub)
                # Bt = rel_ = rel + Bt ;  S = rel_ + time ; D = rel_ - time
                TT(Bt[:], Bt[:], REL, add)
                TT(Sg[:], Bt[:], TM[:], add)
                TT(Dg[:], Bt[:], TM[:], sub)
                # V0 = L0*S0 + TE0*D0 - L1*S1 + TE1*D1
                TT(t0[:], L0, h0(Sg), mult)
                TT(t1[:], h0(TE), h0(Dg), mult)
                TT(h0(V), t0[:], t1[:], add)
                TT(t0[:], L1, h1(Sg), mult)
                TT(h0(V), h0(V), t0[:], sub)
                TT(t1[:], h1(TE), h1(Dg), mult)
                TT(h0(V), h0(V), t1[:], add)
                # V1 = L1*S0 + L0*S1 + TE1*D0 - TE0*D1
                TT(t0[:], L1, h0(Sg), mult)
                TT(t1[:], L0, h1(Sg), mult)
                TT(h1(V), t0[:], t1[:], add)
                TT(t0[:], h1(TE), h0(Dg), mult)
                TT(h1(V), h1(V), t0[:], add)
                TT(t1[:], h0(TE), h1(Dg), mult)
                TT(h1(V), h1(V), t1[:], sub)

                # ---- per-tile: transpose v, cast to bf16, matmul vs E0 shard
                for j in range(GT):
                    bt = g * GT + j
                    vt_ps = pst.tile([P, P], dt.float32, space="PSUM", tag="vtps")
                    nc.tensor.transpose(out=vt_ps[:], in_=V[:, j, :], identity=ident[:])
                    vt = vtp.tile([P, P], dt.bfloat16, tag="vt")
                    nc.scalar.copy(out=vt[:], in_=vt_ps[:])

                    osb = outp.tile([P, NSHARD], dt.float32, tag="osb")
                    for ci, c0 in enumerate(range(0, NSHARD, CHUNK)):
                        cw = min(CHUNK, NSHARD - c0)
                        mm = psm.tile([P, CHUNK], dt.float32, space="PSUM", tag="mm")
                        nc.tensor.matmul(out=mm[:, :cw], lhsT=vt[:],
                                         rhs=e0t[:, c0:c0 + cw],
                                         start=True, stop=True)
                        if ci % 2 == 0:
                            nc.scalar.copy(out=osb[:, c0:c0 + cw], in_=mm[:, :cw])
                        else:
                            nc.vector.tensor_copy(out=osb[:, c0:c0 + cw],
                                                  in_=mm[:, :cw])
                    nc.sync.dma_start(OUT[bt * P:(bt + 1) * P, :], osb[:])

    nc.compile()
    return nc


def _prep_inputs(inputs):
    x = np.asarray(inputs["x"])
    E0 = np.ascontiguousarray(np.asarray(inputs["E0"], dtype=np.float32))
    E1 = np.asarray(inputs["E1"], dtype=np.float32)
    E2 = np.asarray(inputs["E2"], dtype=np.float32)
    E3 = np.asarray(inputs["E3"], dtype=np.float32)
    E4 = np.asarray(inputs["E4"], dtype=np.float32)
    E5 = np.asarray(inputs["E5"], dtype=np.float32)
    E6 = np.asarray(inputs["E6"], dtype=np.float32)
    rule_C = np.asarray(inputs["rule_C"], dtype=np.float32)
    rule_S = np.asarray(inputs["rule_S"], dtype=np.float32)
    has_rules = np.asarray(inputs["has_rules"])

    idx = np.empty((B, 4), np.int32)
    idx[:, 0] = x[:, 0]
    idx[:, 1] = x[:, 1]
    idx[:, 2] = x[:, 3]
    idx[:, 3] = x[:, 3] // CYCLE

    rcat = np.concatenate(
        [E1, rule_C, rule_S[:, None],
         has_rules.astype(np.float32)[:, None]], axis=1).astype(np.float32)
    tcat = np.ascontiguousarray(np.concatenate([E4, E2, E3], axis=1))
    bcat = np.ascontiguousarray(np.concatenate([E5, E6], axis=1))

    in_maps = []
    for i in range(NCORES):
        e0t = np.ascontiguousarray(
            E0[i * NSHARD:(i + 1) * NSHARD, :].T).astype(ml_dtypes.bfloat16)
        in_maps.append({
            "IDX": idx, "E0": E0, "RCAT": rcat, "TCAT": tcat, "BCAT": bcat,
            "E0T": e0t,
        })
    return in_maps


def kernel(**inputs):
    from concourse.bass_utils import run_bass_kernel_spmd

    if "nc" not in _CACHE:
        _CACHE["nc"] = _build()
    nc = _CACHE["nc"]

    in_maps = _prep_inputs(inputs)
    res = run_bass_kernel_spmd(nc, in_maps, core_ids=list(range(NCORES)),
                               trace=TRACE)
    _CACHE["last_result"] = res
    out = np.concatenate([res.results[i]["OUT"] for i in range(NCORES)], axis=1)
    return out


# revision 4
# speedup vs baseline: 1.2375x; 1.2375x over previous
"""Trainium2 Bass kernel for nn_CTRule (temporal KG scoring model).

Computes, for each of B=1024 queries (h, r, t):
  v = f(E0[h], E1[r], time tables, rule tables)   # [B, 128] elementwise algebra
  scores = v @ E0.T                               # [B, 40000]

Distribution over the 8 NeuronCores:
  * The elementwise head is DATA-parallel: core c gathers (indirect DMA) and
    computes v for its own 128-example batch tile only, transposes it on
    TensorE and casts to bf16, then an AllGather shares all vT blocks.
  * The matmul is MODEL-parallel over the 40000-entity axis: each core holds a
    pre-transposed bf16 shard E0T = E0[c*5000:(c+1)*5000].T and emits a
    [1024, 5000] slice of the scores (bf16 on the wire, f32 after the host
    gather).
"""

import numpy as np
import ml_dtypes

P = 128
B = 1024
RANK = 128
NENT = 40000
NREL = 230
NTIME = 365
NBASE = 4
CYCLE = 120
NCORES = 8
NSHARD = NENT // NCORES  # 5000
NT = B // P              # 8 batch tiles
CHUNK = 512              # PSUM bank width in f32

# concatenated-table row widths
RC_W = 2 * RANK + 2      # [E1 | rule_C | rule_S | has_rules] = 258
TC_W = 3 * RANK          # [E4 | E2 | E3] = 384
BC_W = 2 * RANK          # [E5 | E6] = 256

TRACE = False            # set by test harness for profiling runs
_CACHE = {}


def _build():
    import concourse.bass as bass
    import concourse.mybir as mybir
    import concourse.tile as tile
    from concourse import bacc
    from concourse.masks import make_identity

    dt = mybir.dt
    mult = mybir.AluOpType.mult
    add = mybir.AluOpType.add
    sub = mybir.AluOpType.subtract

    nc = bacc.Bacc("TRN2", target_bir_lowering=False, debug=False,
                   num_devices=NCORES)

    IDX = nc.dram_tensor("IDX", [P, 4], dt.int32, kind="ExternalInput").ap()
    E0 = nc.dram_tensor("E0", [NENT, RANK], dt.float32, kind="ExternalInput").ap()
    RCAT = nc.dram_tensor("RCAT", [NREL, RC_W], dt.float32, kind="ExternalInput").ap()
    TCAT = nc.dram_tensor("TCAT", [NTIME, TC_W], dt.float32, kind="ExternalInput").ap()
    BCAT = nc.dram_tensor("BCAT", [NBASE, BC_W], dt.float32, kind="ExternalInput").ap()
    E0T = nc.dram_tensor("E0T", [RANK, NSHARD], dt.bfloat16, kind="ExternalInput").ap()
    OUT = nc.dram_tensor("OUT", [B, NSHARD], dt.bfloat16, kind="ExternalOutput").ap()

    with tile.TileContext(nc) as tc:
        with (
            tc.tile_pool(name="const", bufs=1) as constp,
            tc.tile_pool(name="gath", bufs=1) as gp,
            tc.tile_pool(name="ew", bufs=1) as ew,
            tc.tile_pool(name="outp", bufs=3) as outp,
            tc.tile_pool(name="dram", bufs=1, space="DRAM") as dram,
            tc.tile_pool(name="pst", bufs=1, space="PSUM") as pst,
            tc.tile_pool(name="psm", bufs=6, space="PSUM") as psm,
        ):
            # ---- per-core batch-tile gather (4 indirect DMAs)
            idxt = gp.tile([P, 4], dt.int32)
            nc.sync.dma_start(idxt[:], IDX[:])
            lhs = gp.tile([P, RANK], dt.float32)
            r8 = gp.tile([P, RC_W], dt.float32)
            t8 = gp.tile([P, TC_W], dt.float32)
            b8 = gp.tile([P, BC_W], dt.float32)
            for dst, src, col in (
                (lhs, E0, 0), (r8, RCAT, 1), (t8, TCAT, 2), (b8, BCAT, 3),
            ):
                nc.gpsimd.indirect_dma_start(
                    out=dst[:], out_offset=None, in_=src[:],
                    in_offset=bass.IndirectOffsetOnAxis(
                        ap=idxt[:, col:col + 1], axis=0))

            ident = constp.tile([P, P], dt.float32)
            make_identity(nc, ident[:])
            e0t = constp.tile([RANK, NSHARD], dt.bfloat16)
            nc.sync.dma_start(e0t[:], E0T[:])

            # ---- elementwise head on this core's 128 examples
            REL = r8[:, 0:128]
            RL0, RL1 = r8[:, 0:64], r8[:, 64:128]
            RC0, RC1 = r8[:, 128:192], r8[:, 192:256]
            RS = r8[:, 256:257]
            HR = r8[:, 257:258]
            CT = t8[:, 0:128]
            CT0, CT1 = t8[:, 0:64], t8[:, 64:128]
            E2g, B5 = t8[:, 128:256], b8[:, 0:128]
            E3g, B6 = t8[:, 256:384], b8[:, 128:256]
            LHS = lhs[:, 0:128]
            L0, L1 = lhs[:, 0:64], lhs[:, 64:128]

            TM = ew.tile([P, RANK], dt.float32)
            TE = ew.tile([P, RANK], dt.float32)
            A = ew.tile([P, RANK], dt.float32)
            Bt = ew.tile([P, RANK], dt.float32)
            Sg = ew.tile([P, RANK], dt.float32)
            Dg = ew.tile([P, RANK], dt.float32)
            V = ew.tile([P, RANK], dt.float32)
            t0 = ew.tile([P, 64], dt.float32)
            t1 = ew.tile([P, 64], dt.float32)
            nrs = ew.tile([P, 1], dt.float32)

            def TT(out, a, b_, op):
                nc.vector.tensor_tensor(out=out, in0=a, in1=b_, op=op)

            def h0(x):
                return x[:, 0:64]

            def h1(x):
                return x[:, 64:128]

            # time = E2[t] + E5[tb];  time_ent = E3[t] + E6[tb]
            TT(TM[:], E2g, B5, add)
            TT(TE[:], E3g, B6, add)
            nc.vector.tensor_scalar_mul(nrs[:], RS, -1.0)
            # A = cmul(comp_time, rule_C[r])
            TT(t0[:], CT0, RC0, mult)
            TT(t1[:], CT1, RC1, mult)
            TT(h0(A), t0[:], t1[:], sub)
            TT(t0[:], CT0, RC1, mult)
            TT(t1[:], CT1, RC0, mult)
            TT(h1(A), t0[:], t1[:], add)
            # A = rule_branch = A - rule_S * rel   (fused: A = rel*(-RS) + A)
            nc.vector.scalar_tensor_tensor(
                out=A[:], in0=REL, scalar=nrs[:], in1=A[:], op0=mult, op1=add)
            # Bt = cmul(rel, lhs); then Bt = norule_branch = lhs + Bt
            TT(t0[:], RL0, L0, mult)
            TT(t1[:], RL1, L1, mult)
            TT(h0(Bt), t0[:], t1[:], sub)
            TT(t0[:], RL0, L1, mult)
            TT(t1[:], RL1, L0, mult)
            TT(h1(Bt), t0[:], t1[:], add)
            TT(Bt[:], LHS, Bt[:], add)
            # A = rule_score = Bt + HR*(A - Bt);  then A = q = comp_time + A
            TT(A[:], A[:], Bt[:], sub)
            nc.vector.scalar_tensor_tensor(
                out=A[:], in0=A[:], scalar=HR, in1=Bt[:], op0=mult, op1=add)
            TT(A[:], A[:], CT, add)
            # Bt = complex_mul(rel, q) = [R0*q0 + R1*q1, R0*q1 - R1*q0]
            TT(t0[:], RL0, h0(A), mult)
            TT(t1[:], RL1, h1(A), mult)
            TT(h0(Bt), t0[:], t1[:], add)
            TT(t0[:], RL0, h1(A), mult)
            TT(t1[:], RL1, h0(A), mult)
            TT(h1(Bt), t0[:], t1[:], sub)
            # Bt = rel_ = rel + Bt ;  S = rel_ + time ; D = rel_ - time
            TT(Bt[:], Bt[:], REL, add)
            TT(Sg[:], Bt[:], TM[:], add)
            TT(Dg[:], Bt[:], TM[:], sub)
            # V0 = L0*S0 + TE0*D0 - L1*S1 + TE1*D1
            TT(t0[:], L0, h0(Sg), mult)
            TT(t1[:], h0(TE), h0(Dg), mult)
            TT(h0(V), t0[:], t1[:], add)
            TT(t0[:], L1, h1(Sg), mult)
            TT(h0(V), h0(V), t0[:], sub)
            TT(t1[:], h1(TE), h1(Dg), mult)
            TT(h0(V), h0(V), t1[:], add)
            # V1 = L1*S0 + L0*S1 + TE1*D0 - TE0*D1
            TT(t0[:], L1, h0(Sg), mult)
            TT(t1[:], L0, h1(Sg), mult)
            TT(h1(V), t0[:], t1[:], add)
            TT(t0[:], h1(TE), h0(Dg), mult)
            TT(h1(V), h1(V), t0[:], add)
            TT(t1[:], h0(TE), h1(Dg), mult)
            TT(h1(V), h1(V), t1[:], sub)

            # ---- transpose + bf16 cast + AllGather of vT across the 8 cores
            vt_ps = pst.tile([P, P], dt.float32, space="PSUM")
            nc.tensor.transpose(out=vt_ps[:], in_=V[:], identity=ident[:])
            vt = constp.tile([P, P], dt.bfloat16)
            nc.scalar.copy(out=vt[:], in_=vt_ps[:])

            ag_in = dram.tile([P, P], dt.bfloat16)
            ag_out = dram.tile([NCORES * P, P], dt.bfloat16, addr_space="Shared")
            nc.sync.dma_start(ag_in[:], vt[:])
            nc.gpsimd.collective_compute(
                "AllGather",
                mybir.AluOpType.bypass,
                replica_groups=[list(range(NCORES))],
                ins=[ag_in[:]],
                outs=[ag_out[:]],
            )
            vt_all = constp.tile([P, NT, P], dt.bfloat16)
            for j in range(NT):
                nc.sync.dma_start(vt_all[:, j, :], ag_out[j * P:(j + 1) * P, :])

            # ---- model-parallel matmul: all 8 vT blocks vs this core's shard
            for j in range(NT):
                osb = outp.tile([P, NSHARD], dt.bfloat16, tag="osb")
                for ci, c0 in enumerate(range(0, NSHARD, CHUNK)):
                    cw = min(CHUNK, NSHARD - c0)
                    mm = psm.tile([P, CHUNK], dt.float32, space="PSUM", tag="mm")
                    nc.tensor.matmul(out=mm[:, :cw], lhsT=vt_all[:, j, :],
                                     rhs=e0t[:, c0:c0 + cw],
                                     start=True, stop=True)
                    if ci % 2 == 0:
                        nc.scalar.copy(out=osb[:, c0:c0 + cw], in_=mm[:, :cw])
                    else:
                        nc.vector.tensor_copy(out=osb[:, c0:c0 + cw],
                                              in_=mm[:, :cw])
                nc.sync.dma_start(OUT[j * P:(j + 1) * P, :], osb[:])

    nc.compile()
    return nc


def _prep_inputs(inputs):
    x = np.asarray(inputs["x"])
    E0 = np.ascontiguousarray(np.asarray(inputs["E0"], dtype=np.float32))
    E1 = np.asarray(inputs["E1"], dtype=np.float32)
    E2 = np.asarray(inputs["E2"], dtype=np.float32)
    E3 = np.asarray(inputs["E3"], dtype=np.float32)
    E4 = np.asarray(inputs["E4"], dtype=np.float32)
    E5 = np.asarray(inputs["E5"], dtype=np.float32)
    E6 = np.asarray(inputs["E6"], dtype=np.float32)
    rule_C = np.asarray(inputs["rule_C"], dtype=np.float32)
    rule_S = np.asarray(inputs["rule_S"], dtype=np.float32)
    has_rules = np.asarray(inputs["has_rules"])

    idx = np.empty((B, 4), np.int32)
    idx[:, 0] = x[:, 0]
    idx[:, 1] = x[:, 1]
    idx[:, 2] = x[:, 3]
    idx[:, 3] = x[:, 3] // CYCLE

    rcat = np.concatenate(
        [E1, rule_C, rule_S[:, None],
         has_rules.astype(np.float32)[:, None]], axis=1).astype(np.float32)
    tcat = np.ascontiguousarray(np.concatenate([E4, E2, E3], axis=1))
    bcat = np.ascontiguousarray(np.concatenate([E5, E6], axis=1))

    in_maps = []
    for i in range(NCORES):
        e0t = np.ascontiguousarray(
            E0[i * NSHARD:(i + 1) * NSHARD, :].T).astype(ml_dtypes.bfloat16)
        in_maps.append({
            "IDX": idx[i * P:(i + 1) * P], "E0": E0, "RCAT": rcat,
            "TCAT": tcat, "BCAT": bcat, "E0T": e0t,
        })
    return in_maps


def kernel(**inputs):
    from concourse.bass_utils import run_bass_kernel_spmd

    if "nc" not in _CACHE:
        _CACHE["nc"] = _build()
    nc = _CACHE["nc"]

    in_maps = _prep_inputs(inputs)
    res = run_bass_kernel_spmd(nc, in_maps, core_ids=list(range(NCORES)),
                               trace=TRACE)
    _CACHE["last_result"] = res
    out = np.concatenate(
        [res.results[i]["OUT"].astype(np.float32) for i in range(NCORES)],
        axis=1)
    return out


# revision 5
# speedup vs baseline: 1.9428x; 1.5700x over previous
"""Trainium2 Bass kernel for nn_CTRule (temporal KG scoring model).

Computes, for each of B=1024 queries (h, r, t):
  v = f(E0[h], E1[r], time tables, rule tables)   # [B, 128] elementwise algebra
  scores = v @ E0.T                               # [B, 40000]

Distribution over the 8 NeuronCores: fully data-parallel over the batch.
Core c owns batch rows [c*128, (c+1)*128):
  * gathers its examples' embedding rows with indirect DMA and runs the
    elementwise head on VectorE,
  * transposes v on TensorE and casts to bf16,
  * streams the full pre-transposed bf16 entity table E0T [128, 40000] from
    HBM through SBUF and matmuls against it chunk by chunk,
  * writes its [128, 40000] bf16 row-block of the scores.
No cross-core communication; the host stacks the 8 row-blocks and casts f32.
"""

import numpy as np
import ml_dtypes

P = 128
B = 1024
RANK = 128
NENT = 40000
NREL = 230
NTIME = 365
NBASE = 4
CYCLE = 120
NCORES = 8
NT = B // P              # 8 batch tiles == cores
CHUNK = 512              # PSUM bank width in f32
LOADCH = 2500            # E0T load-chunk columns (16 loads of 640KB)
OUTCH = 5000             # output DMA chunk columns (8 DMAs of 1.25MB)

# concatenated-table row widths
RC_W = 2 * RANK + 2      # [E1 | rule_C | rule_S | has_rules] = 258
TC_W = 3 * RANK          # [E4 | E2 | E3] = 384
BC_W = 2 * RANK          # [E5 | E6] = 256

TRACE = False            # set by test harness for profiling runs
_CACHE = {}


def _build():
    import concourse.bass as bass
    import concourse.mybir as mybir
    import concourse.tile as tile
    from concourse import bacc
    from concourse.masks import make_identity

    dt = mybir.dt
    mult = mybir.AluOpType.mult
    add = mybir.AluOpType.add
    sub = mybir.AluOpType.subtract

    nc = bacc.Bacc("TRN2", target_bir_lowering=False, debug=False,
                   num_devices=NCORES)

    IDX = nc.dram_tensor("IDX", [P, 4], dt.int32, kind="ExternalInput").ap()
    E0 = nc.dram_tensor("E0", [NENT, RANK], dt.float32, kind="ExternalInput").ap()
    RCAT = nc.dram_tensor("RCAT", [NREL, RC_W], dt.float32, kind="ExternalInput").ap()
    TCAT = nc.dram_tensor("TCAT", [NTIME, TC_W], dt.float32, kind="ExternalInput").ap()
    BCAT = nc.dram_tensor("BCAT", [NBASE, BC_W], dt.float32, kind="ExternalInput").ap()
    E0T = nc.dram_tensor("E0T", [RANK, NENT], dt.bfloat16, kind="ExternalInput").ap()
    OUT = nc.dram_tensor("OUT", [P, NENT], dt.bfloat16, kind="ExternalOutput").ap()

    with tile.TileContext(nc) as tc:
        with (
            tc.tile_pool(name="const", bufs=1) as constp,
            tc.tile_pool(name="gath", bufs=1) as gp,
            tc.tile_pool(name="ew", bufs=1) as ew,
            tc.tile_pool(name="pst", bufs=1, space="PSUM") as pst,
            tc.tile_pool(name="psm", bufs=6, space="PSUM") as psm,
        ):
            # ---- per-core batch-tile gather (4 indirect DMAs)
            idxt = gp.tile([P, 4], dt.int32)
            nc.sync.dma_start(idxt[:], IDX[:])
            lhs = gp.tile([P, RANK], dt.float32)
            r8 = gp.tile([P, RC_W], dt.float32)
            t8 = gp.tile([P, TC_W], dt.float32)
            b8 = gp.tile([P, BC_W], dt.float32)
            for dst, src, col in (
                (lhs, E0, 0), (r8, RCAT, 1), (t8, TCAT, 2), (b8, BCAT, 3),
            ):
                nc.gpsimd.indirect_dma_start(
                    out=dst[:], out_offset=None, in_=src[:],
                    in_offset=bass.IndirectOffsetOnAxis(
                        ap=idxt[:, col:col + 1], axis=0))

            # ---- stream the full transposed entity table into SBUF
            e0t = constp.tile([RANK, NENT], dt.bfloat16)
            for c0 in range(0, NENT, LOADCH):
                nc.sync.dma_start(e0t[:, c0:c0 + LOADCH], E0T[:, c0:c0 + LOADCH])

            ident = constp.tile([P, P], dt.float32)
            make_identity(nc, ident[:])

            # ---- elementwise head on this core's 128 examples
            REL = r8[:, 0:128]
            RL0, RL1 = r8[:, 0:64], r8[:, 64:128]
            RC0, RC1 = r8[:, 128:192], r8[:, 192:256]
            RS = r8[:, 256:257]
            HR = r8[:, 257:258]
            CT = t8[:, 0:128]
            CT0, CT1 = t8[:, 0:64], t8[:, 64:128]
            E2g, B5 = t8[:, 128:256], b8[:, 0:128]
            E3g, B6 = t8[:, 256:384], b8[:, 128:256]
            LHS = lhs[:, 0:128]
            L0, L1 = lhs[:, 0:64], lhs[:, 64:128]

            TM = ew.tile([P, RANK], dt.float32)
            TE = ew.tile([P, RANK], dt.float32)
            A = ew.tile([P, RANK], dt.float32)
            Bt = ew.tile([P, RANK], dt.float32)
            Sg = ew.tile([P, RANK], dt.float32)
            Dg = ew.tile([P, RANK], dt.float32)
            V = ew.tile([P, RANK], dt.float32)
            t0 = ew.tile([P, 64], dt.float32)
            t1 = ew.tile([P, 64], dt.float32)
            nrs = ew.tile([P, 1], dt.float32)

            def TT(out, a, b_, op):
                nc.vector.tensor_tensor(out=out, in0=a, in1=b_, op=op)

            def h0(x):
                return x[:, 0:64]

            def h1(x):
                return x[:, 64:128]

            # time = E2[t] + E5[tb];  time_ent = E3[t] + E6[tb]
            TT(TM[:], E2g, B5, add)
            TT(TE[:], E3g, B6, add)
            nc.vector.tensor_scalar_mul(nrs[:], RS, -1.0)
            # A = cmul(comp_time, rule_C[r])
            TT(t0[:], CT0, RC0, mult)
            TT(t1[:], CT1, RC1, mult)
            TT(h0(A), t0[:], t1[:], sub)
            TT(t0[:], CT0, RC1, mult)
            TT(t1[:], CT1, RC0, mult)
            TT(h1(A), t0[:], t1[:], add)
            # A = rule_branch = A - rule_S * rel   (fused: A = rel*(-RS) + A)
            nc.vector.scalar_tensor_tensor(
                out=A[:], in0=REL, scalar=nrs[:], in1=A[:], op0=mult, op1=add)
            # Bt = cmul(rel, lhs); then Bt = norule_branch = lhs + Bt
            TT(t0[:], RL0, L0, mult)
            TT(t1[:], RL1, L1, mult)
            TT(h0(Bt), t0[:], t1[:], sub)
            TT(t0[:], RL0, L1, mult)
            TT(t1[:], RL1, L0, mult)
            TT(h1(Bt), t0[:], t1[:], add)
            TT(Bt[:], LHS, Bt[:], add)
            # A = rule_score = Bt + HR*(A - Bt);  then A = q = comp_time + A
            TT(A[:], A[:], Bt[:], sub)
            nc.vector.scalar_tensor_tensor(
                out=A[:], in0=A[:], scalar=HR, in1=Bt[:], op0=mult, op1=add)
            TT(A[:], A[:], CT, add)
            # Bt = complex_mul(rel, q) = [R0*q0 + R1*q1, R0*q1 - R1*q0]
            TT(t0[:], RL0, h0(A), mult)
            TT(t1[:], RL1, h1(A), mult)
            TT(h0(Bt), t0[:], t1[:], add)
            TT(t0[:], RL0, h1(A), mult)
            TT(t1[:], RL1, h0(A), mult)
            TT(h1(Bt), t0[:], t1[:], sub)
            # Bt = rel_ = rel + Bt ;  S = rel_ + time ; D = rel_ - time
            TT(Bt[:], Bt[:], REL, add)
            TT(Sg[:], Bt[:], TM[:], add)
            TT(Dg[:], Bt[:], TM[:], sub)
            # V0 = L0*S0 + TE0*D0 - L1*S1 + TE1*D1
            TT(t0[:], L0, h0(Sg), mult)
            TT(t1[:], h0(TE), h0(Dg), mult)
            TT(h0(V), t0[:], t1[:], add)
            TT(t0[:], L1, h1(Sg), mult)
            TT(h0(V), h0(V), t0[:], sub)
            TT(t1[:], h1(TE), h1(Dg), mult)
            TT(h0(V), h0(V), t1[:], add)
            # V1 = L1*S0 + L0*S1 + TE1*D0 - TE0*D1
            TT(t0[:], L1, h0(Sg), mult)
            TT(t1[:], L0, h1(Sg), mult)
            TT(h1(V), t0[:], t1[:], add)
            TT(t0[:], h1(TE), h0(Dg), mult)
            TT(h1(V), h1(V), t0[:], add)
            TT(t1[:], h0(TE), h1(Dg), mult)
            TT(h1(V), h1(V), t1[:], sub)

            # ---- transpose + bf16 cast of vT (the stationary matmul operand)
            vt_ps = pst.tile([P, P], dt.float32, space="PSUM")
            nc.tensor.transpose(out=vt_ps[:], in_=V[:], identity=ident[:])
            vt = constp.tile([P, P], dt.bfloat16)
            nc.scalar.copy(out=vt[:], in_=vt_ps[:])

            # ---- stream matmuls over the full entity axis
            osb = constp.tile([P, NENT], dt.bfloat16)
            next_out = OUTCH
            for ci, c0 in enumerate(range(0, NENT, CHUNK)):
                cw = min(CHUNK, NENT - c0)
                mm = psm.tile([P, CHUNK], dt.float32, space="PSUM", tag="mm")
                nc.tensor.matmul(out=mm[:, :cw], lhsT=vt[:],
                                 rhs=e0t[:, c0:c0 + cw],
                                 start=True, stop=True)
                if ci % 2 == 0:
                    nc.scalar.copy(out=osb[:, c0:c0 + cw], in_=mm[:, :cw])
                else:
                    nc.vector.tensor_copy(out=osb[:, c0:c0 + cw], in_=mm[:, :cw])
                if c0 + cw >= next_out:
                    o0 = next_out - OUTCH
                    nc.sync.dma_start(OUT[:, o0:next_out], osb[:, o0:next_out])
                    next_out += OUTCH

    nc.compile()
    return nc


def _prep_inputs(inputs):
    x = np.asarray(inputs["x"])
    E0 = np.ascontiguousarray(np.asarray(inputs["E0"], dtype=np.float32))
    E1 = np.asarray(inputs["E1"], dtype=np.float32)
    E2 = np.asarray(inputs["E2"], dtype=np.float32)
    E3 = np.asarray(inputs["E3"], dtype=np.float32)
    E4 = np.asarray(inputs["E4"], dtype=np.float32)
    E5 = np.asarray(inputs["E5"], dtype=np.float32)
    E6 = np.asarray(inputs["E6"], dtype=np.float32)
    rule_C = np.asarray(inputs["rule_C"], dtype=np.float32)
    rule_S = np.asarray(inputs["rule_S"], dtype=np.float32)
    has_rules = np.asarray(inputs["has_rules"])

    idx = np.empty((B, 4), np.int32)
    idx[:, 0] = x[:, 0]
    idx[:, 1] = x[:, 1]
    idx[:, 2] = x[:, 3]
    idx[:, 3] = x[:, 3] // CYCLE

    rcat = np.concatenate(
        [E1, rule_C, rule_S[:, None],
         has_rules.astype(np.float32)[:, None]], axis=1).astype(np.float32)
    tcat = np.ascontiguousarray(np.concatenate([E4, E2, E3], axis=1))
    bcat = np.ascontiguousarray(np.concatenate([E5, E6], axis=1))
    e0t = np.ascontiguousarray(E0.T).astype(ml_dtypes.bfloat16)

    in_maps = []
    for i in range(NCORES):
        in_maps.append({
            "IDX": idx[i * P:(i + 1) * P], "E0": E0, "RCAT": rcat,
            "TCAT": tcat, "BCAT": bcat, "E0T": e0t,
        })
    return in_maps


def kernel(**inputs):
    from concourse.bass_utils import run_bass_kernel_spmd

    if "nc" not in _CACHE:
        _CACHE["nc"] = _build()
    nc = _CACHE["nc"]

    in_maps = _prep_inputs(inputs)
    res = run_bass_kernel_spmd(nc, in_maps, core_ids=list(range(NCORES)),
                               trace=TRACE)
    _CACHE["last_result"] = res
    out = np.concatenate(
        [res.results[i]["OUT"] for i in range(NCORES)],
        axis=0).astype(np.float32)
    return out
